# revision 3
# baseline (speedup 1.0000x reference)
"""Trainium2 Bass kernel for nn_DotProductAttention_76338748719461.

Attention with a multiplicative mask and softmax over the QUERY axis
(axis=1 of [B, Lq, Lk] scores):

    S[b,q,k]  = (Q[b,q,:] . K[b,k,:]) / 8 + max(log(mask[0,q,k]), F32_MIN)
    A         = softmax(S, axis=q)
    out[b,q,v]= sum_k A[b,q,k] * V[b,k,v]

Key identity: exp(S + log m) = exp(S) * m, so the mask is applied as a
multiply after exp — no log, no additive bias, and mask==0 handled exactly.

Strategy (per NeuronCore; batch is data-parallel over 8 cores, 2 per core).
ACT (scalar engine) is the inescapable bottleneck — 65536 exp elements per
partition per batch-pair at ~1 elem/cycle — so every other piece of work is
kept off ACT and under its ~66us of busy time:

  * Work in the TRANSPOSED score orientation S_T[k, q] so the softmax
    reduction (over q) is a free-axis reduction.
  * ALL transposes (mask AND Q/K) are done with ZERO compute-engine work:
    f32->f16 cast DMAs into DRAM scratches (SWDGE/gpsimd queue), then the
    hardware xbar DMA-transpose into SBUF (SP queue). Q and K ride as the
    column-halves of [Q|K] and [K|Q] scratches so that Q^T and K^T both
    land at partition base 0.
  * S_T = KT.T @ QT in f16 (1 cyc/row), exp on ACT with the 1/8 scale
    folded into the activation's scale immediate (PSUM -> SBUF f16, no
    accumulator).
  * Mask multiply runs in-place on the exp output, split across engines:
    half on DVE tensor_tensor (f16 2x mode), half on Pool/gpsimd
    tensor_tensor. Softmax denominators come from DVE tensor_scalar
    (x1.0 + accum_out row-sum), which runs in 4x mode.
  * out_T[v, q] = sum_k (V[k,v]/D[k]) . PM_T[k, q] via f16 PE
    accumulation, then PE-transposed back to [q, v] and DMA'd out.
"""

import os
import numpy as np

B, LQ, LK, D, DV = 16, 2048, 2048, 64, 64
NCORES = 8
BPC = B // NCORES  # batches per core
P = 128
CH = 512  # matmul moving chunk (one PSUM bank of fp32)
HALF = 1024  # exp / multiply granularity (half a k-tile row)
NT_Q = LQ // P  # 16
NT_K = LK // P  # 16
SCALE = 1.0 / 8.0  # 1/sqrt(64)

MAIN_REPS = int(os.environ.get("MAIN_REPS", "1"))  # repeat body (timing builds)

_CACHED = None


def _emit_body(nc, tc, ctx, aps, dts):
    import concourse.mybir as mybir
    from concourse.bass import ds
    from concourse.masks import make_identity

    (q_d, k_d, v_d, m_d, m16_d, qkA_d, qkB_d, o_d) = aps
    f32, f16, AF = dts

    consts = ctx.enter_context(tc.tile_pool(name="consts", bufs=1))
    ident32 = consts.tile([P, P], f32)
    make_identity(nc, ident32)

    big = ctx.enter_context(tc.tile_pool(name="big", bufs=1))
    maskT = big.tile([P, NT_K, LQ], f16)  # mask[q,k].T
    QKA = big.tile([P, BPC, LQ], f16)  # [0:64] = Q^T (unscaled)
    QKB = big.tile([P, BPC, LK], f16)  # [0:64] = K^T
    v_nat = big.tile([P, BPC, NT_K, DV], f32)

    # f32->f16 cast DMAs into DRAM scratches (SWDGE queue, in order):
    # [Q|K] and [K|Q] per batch-0 first, then batch 1 later (emitted below).
    def emit_qk_cast(b):
        nc.gpsimd.dma_start(qkA_d[b, :, 0:D], q_d[b])
        nc.gpsimd.dma_start(qkA_d[b, :, D : 2 * D], k_d[b])
        nc.gpsimd.dma_start(qkB_d[b, :, 0:D], k_d[b])
        nc.gpsimd.dma_start(qkB_d[b, :, D : 2 * D], q_d[b])

    def emit_qk_transpose(b):
        nc.sync.dma_start(QKA[:, b, :], qkA_d[b], transpose=True)
        nc.sync.dma_start(QKB[:, b, :], qkB_d[b], transpose=True)

    emit_qk_cast(0)
    emit_qk_transpose(0)

    # ---------- main pools ----------
    psum_s = ctx.enter_context(tc.tile_pool(name="psum_s", bufs=2, space="PSUM"))
    psum_o = ctx.enter_context(tc.tile_pool(name="psum_o", bufs=1, space="PSUM"))
    work = ctx.enter_context(tc.tile_pool(name="work", bufs=3))
    outp = ctx.enter_context(tc.tile_pool(name="outp", bufs=2))

    for _mr in range(MAIN_REPS):
        for b in range(BPC):
            _emit_batch(
                nc, tc, aps, dts, ident32, maskT, QKA, QKB, v_nat,
                psum_s, psum_o, work, outp, b,
                mask_prep=(b == 0),
                next_prep=(emit_qk_cast, emit_qk_transpose)
                if (b == 0 and _mr == 0)
                else None,
            )


def _emit_batch(nc, tc, aps, dts, ident32, maskT, QKA, QKB, v_nat,
                psum_s, psum_o, work, outp, b, mask_prep, next_prep):
    import concourse.mybir as mybir
    from concourse.bass import ds, ts

    (q_d, k_d, v_d, m_d, m16_d, qkA_d, qkB_d, o_d) = aps
    f32, f16, AF = dts
    ALU = mybir.AluOpType

    O_ps = psum_o.tile([DV, LQ], f32, tag="o", name="O_ps")
    pending_av = None

    if mask_prep:
        # f32->f16 cast DMAs to DRAM scratch, ramped widths: narrow first
        # (pipeline startup latency), wide later (better DMA efficiency).
        edge = 0
        for w in (P, P, P, P, 4 * P, 4 * P, 4 * P):
            nc.gpsimd.dma_start(m16_d[:, ds(edge, w)], m_d[:, ds(edge, w)])
            edge += w
        assert edge == LK
        # V loads ride the SWDGE queue behind the casts
        nc.gpsimd.dma_start(
            v_nat[:], v_d.rearrange("b (t p) d -> p b t d", p=P)
        )

    for j in range(NT_K):
        if mask_prep:
            # xbar DMA-transpose of column-panel j into SBUF (SP queue)
            nc.sync.dma_start(
                maskT[:, j, :], m16_d[:, ds(P * j, P)], transpose=True
            )

        Sh = [
            psum_s.tile([P, HALF], f32, tag="s", name=f"s{h}")
            for h in range(2)
        ]
        for h in range(2):
            for c in range(2):
                nc.tensor.matmul(
                    Sh[h][:, ts(c, CH)],
                    QKB[0:D, b, ds(P * j, P)],
                    QKA[0:D, b, ds(HALF * h + CH * c, CH)],
                    start=True,
                    stop=True,
                )

        # deferred AV of previous k-tile keeps PE busy while exp runs
        if pending_av is not None:
            _emit_av(nc, O_ps, pending_av)

        E = work.tile([P, LQ], f16, tag="e", name="E")
        D2 = work.tile([P, 2], f32, tag="d2", name="D2")
        for h in range(2):
            hs = ds(HALF * h, HALF)
            nc.scalar.activation(E[:, hs], Sh[h][:], AF.Exp, scale=SCALE)
        # in-place mask multiply: half 0 on DVE, half 1 on Pool/gpsimd
        h0 = ds(0, HALF)
        h1 = ds(HALF, HALF)
        nc.vector.tensor_tensor(E[:, h0], E[:, h0], maskT[:, j, h0], ALU.mult)
        nc.gpsimd.tensor_tensor(E[:, h1], E[:, h1], maskT[:, j, h1], ALU.mult)
        # row-sums via tensor_scalar(x1+0) accum (4x DVE mode)
        for h, hsl in ((0, h0), (1, h1)):
            nc.vector.tensor_scalar(
                out=E[:, hsl], in0=E[:, hsl], scalar1=1.0, scalar2=0.0,
                op0=ALU.mult, op1=ALU.add, accum_out=D2[:, ds(h, 1)],
            )
        Dsum = work.tile([P, 1], f32, tag="dsum", name="Dsum")
        nc.vector.reduce_sum(Dsum[:], D2[:], axis=mybir.AxisListType.X)
        R = work.tile([P, 1], f32, tag="r", name="R")
        nc.vector.reciprocal(R[:], Dsum[:])
        Vp = work.tile([P, DV], f16, tag="vp", name="Vp")
        nc.vector.tensor_scalar_mul(Vp[:], v_nat[:, b, j, :], R[:])
        pending_av = (Vp, E, j)

    _emit_av(nc, O_ps, pending_av)

    if next_prep is not None:
        # batch 1's Q/K casts + xbar transposes, queued behind batch 0's
        # mask panels on their respective DMA queues
        cast, transpose = next_prep
        cast(1)
        transpose(1)

    # evacuate + transpose back to [q, v]
    OT = outp.tile([DV, LQ], f32, tag="ot", name="OT")
    nc.vector.tensor_copy(OT[:], O_ps[:])
    out_sb = outp.tile([P, NT_Q, DV], f32, tag="osb", name="out_sb")
    for g in range(NT_Q // 8):
        tp = psum_o.tile([P, 8 * DV], f32, tag="o", name="tp")
        for u in range(8):
            t = 8 * g + u
            nc.tensor.transpose(
                tp[:, ds(DV * u, DV)],
                OT[:, ds(P * t, P)],
                ident32[0:DV, 0:DV],
            )
        nc.vector.tensor_copy(
            out_sb[:, ds(8 * g, 8), :],
            tp[:].rearrange("p (t d) -> p t d", d=DV),
        )
    nc.sync.dma_start(o_d[b].rearrange("(t p) d -> p t d", p=P), out_sb[:])


def _emit_av(nc, O_ps, pending):
    from concourse.bass import ts

    pVp, pE, pj = pending
    for c in range(LQ // CH):
        nc.tensor.matmul(
            O_ps[:, ts(c, CH)],
            pVp[:],
            pE[:, ts(c, CH)],
            start=(pj == 0),
            stop=(pj == NT_K - 1),
        )


def _build_module():
    import concourse.mybir as mybir
    import concourse.tile as tile
    from concourse import bacc
    from contextlib import ExitStack

    f32 = mybir.dt.float32
    f16 = mybir.dt.float16
    dts = (f32, f16, mybir.ActivationFunctionType)

    nc = bacc.Bacc("TRN2", target_bir_lowering=False, debug=False)
    q_d = nc.dram_tensor("q", [BPC, LQ, D], f32, kind="ExternalInput").ap()
    k_d = nc.dram_tensor("k", [BPC, LK, D], f32, kind="ExternalInput").ap()
    v_d = nc.dram_tensor("v", [BPC, LK, DV], f32, kind="ExternalInput").ap()
    m_d = nc.dram_tensor("m", [LQ, LK], f32, kind="ExternalInput").ap()
    m16_d = nc.dram_tensor("m16", [LQ, LK], f16, kind="Internal").ap()
    qkA_d = nc.dram_tensor("qkA", [BPC, LQ, 2 * D], f16, kind="Internal").ap()
    qkB_d = nc.dram_tensor("qkB", [BPC, LK, 2 * D], f16, kind="Internal").ap()
    o_d = nc.dram_tensor("o", [BPC, LQ, DV], f32, kind="ExternalOutput").ap()
    aps = (q_d, k_d, v_d, m_d, m16_d, qkA_d, qkB_d, o_d)

    with tile.TileContext(nc) as tc:
        with ExitStack() as rctx:
            _emit_body(nc, tc, rctx, aps, dts)

    nc.compile()
    return nc


def _get_module():
    global _CACHED
    if _CACHED is None:
        _CACHED = _build_module()
    return _CACHED


def kernel(query, key, value, mask, _trace=False):
    from concourse.bass_utils import run_bass_kernel_spmd

    query = np.asarray(query, dtype=np.float32)
    key = np.asarray(key, dtype=np.float32)
    value = np.asarray(value, dtype=np.float32)
    mask = np.asarray(mask, dtype=np.float32)

    nc = _get_module()
    in_maps = [
        {
            "q": query[c * BPC : (c + 1) * BPC],
            "k": key[c * BPC : (c + 1) * BPC],
            "v": value[c * BPC : (c + 1) * BPC],
            "m": mask[0],
        }
        for c in range(NCORES)
    ]
    res = run_bass_kernel_spmd(
        nc, in_maps, core_ids=list(range(NCORES)), trace=_trace
    )
    out = np.concatenate([res.results[c]["o"] for c in range(NCORES)], axis=0)
    if _trace:
        return out, res
    return out
